# revision 4
# baseline (speedup 1.0000x reference)
"""Trainium2 Bass kernel for nn_ContactMapDistError.

Computes, for each batch element b:
    mean over active contact pairs (r,s) of
      min_{v in region r, w in region s} || g1[b,r,v] - g2[b,s,w] ||

Strategy (v2 — full on-device reduction, scalar output per core)
---------------------------------------------------------------
Host (cheap, O(B*R*VR)):
  - gather region vertex subsets g1, g2 via rid_to_vid
  - build fp16 feature matrices so a single K=5 matmul produces the full
    pairwise squared-distance matrix:
        d2(v,w) = [-2x,-2y,-2z,sq1,1]_v . [x',y',z',1,sq2]_w
    (fp16 inputs ~ same precision as the fp32r path: PSUM accumulates f32)
  - mask input [48, 25]: cols 0-23 = cmap slice, col 24 = ones (for the
    final partition-sum matmul)

Device (8 cores, SPMD; core i -> batch i//2, r-half i%2), raw bass:
  - PE: 54 tiles x 3 matmuls [5,128]x[5,512] -> d2 in PSUM [128,1536]
  - DVE: grouped min over each s-region's 96 w columns -> m1 [128, 18*48]
  - PE: 18 transposes (identity built on Pool via iota+is_equal) to flip
    the packed v axis into the free dimension -> mts [48, 2304]
  - DVE: grouped min over each r-region's 96 v columns -> md2 [48, 24]
  - DVE: relu * mask; ACT: sqrt with free-axis accumulate -> sums [48,1]
  - PE: partition-sum matmul (ones column) -> scalar; DMA out [1,1] f32
"""

import sys

sys.path.insert(0, "/opt/trn_rl_repo")

import numpy as np

import concourse.bass as bass
import concourse.mybir as mybir
from concourse.bass_utils import run_bass_kernel_spmd

F16 = mybir.dt.float16
F32 = mybir.dt.float32
I32 = mybir.dt.int32

B, N, R, VR = 4, 10475, 48, 96
NCORES = 8
RH = R // 2            # r-regions handled per core
V = RH * VR            # packed v columns per core = 2304
T = V // 128           # v-chunks of 128 partitions = 18
W = R * VR             # full w width = 4608
WC = 1536              # psum w-chunk (3 banks, 16 s-regions)
NWC = W // WC          # = 3
K = 5                  # contraction dim
NK = T * NWC           # stage-1 tile count = 54

_cache = {}


def _build():
    if "nc" in _cache:
        return _cache["nc"]
    nc = bass.Bass()
    ab = nc.declare_dram_parameter("ab", [K, V + W], F16, isOutput=False)
    maskin = nc.declare_dram_parameter("maskin", [R, RH + 1], F32, isOutput=False)
    res_out = nc.declare_dram_parameter("res_out", [1, 1], F32, isOutput=True)

    abt = nc.alloc_sbuf_tensor("abt", [K, V + W], F16).ap()
    maskt = nc.alloc_sbuf_tensor("maskt", [R, RH + 1], F32).ap()
    iot = nc.alloc_sbuf_tensor("iot", [128, 128], I32).ap()
    idf32 = nc.alloc_sbuf_tensor("idf32", [128, 128], F32).ap()
    m1 = nc.alloc_sbuf_tensor("m1", [128, T * R], F32).ap()
    mts = nc.alloc_sbuf_tensor("mts", [R, V], F32).ap()
    md2 = nc.alloc_sbuf_tensor("md2", [R, RH], F32).ap()
    md2m = nc.alloc_sbuf_tensor("md2m", [R, RH], F32).ap()
    sq48 = nc.alloc_sbuf_tensor("sq48", [R, RH], F32).ap()
    sums = nc.alloc_sbuf_tensor("sums", [R, 1], F32).ap()
    res = nc.alloc_sbuf_tensor("res", [1, 1], F32).ap()

    pts = [nc.alloc_psum_tensor(f"pt{i}", [128, WC], F32).ap() for i in range(2)]
    tscr = [nc.alloc_psum_tensor(f"ts{i}", [R, 512], F32).ap() for i in range(2)]

    lt = abt[:, 0:V]          # v-side features (stationary)
    rt = abt[:, V : V + W]    # w-side features (moving)
    ones = maskt[:, RH : RH + 1]

    with (
        nc.Block() as block,
        nc.semaphore("dma_sem") as dma_sem,
        nc.semaphore("pe_sem") as pe_sem,
        nc.semaphore("dve_sem") as dve_sem,
        nc.semaphore("act_sem") as act_sem,
        nc.semaphore("id_sem") as id_sem,
    ):

        @block.sync
        def _(sp):
            sp.dma_start(abt, ab[:]).then_inc(dma_sem, 16)
            sp.dma_start(maskt, maskin[:]).then_inc(dma_sem, 16)
            sp.wait_ge(act_sem, 7)
            sp.dma_start(res_out[:], res).then_inc(dma_sem, 16)
            sp.wait_ge(dma_sem, 48)

        @block.gpsimd
        def _(g):
            # identity matrix for PE transposes: (j - p == 0)
            g.iota(
                iot,
                [[1, 128]],
                base=0,
                channel_multiplier=-1,
                allow_small_or_imprecise_dtypes=True,
            )
            g.drain()
            g.tensor_scalar(
                idf32, iot, 0, None, op0=mybir.AluOpType.is_equal
            ).then_inc(id_sem)

        @block.tensor
        def _(pe):
            pe.wait_ge(dma_sem, 32)
            # stage 1: d2 tiles
            for k in range(NK):
                t, c = divmod(k, NWC)
                if k >= 2:
                    pe.wait_ge(dve_sem, k - 1)
                pt = pts[k % 2]
                last = None
                for m in range(WC // 512):
                    last = pe.matmul(
                        pt[:, m * 512 : (m + 1) * 512],
                        lt[:, t * 128 : (t + 1) * 128],
                        rt[:, c * WC + m * 512 : c * WC + (m + 1) * 512],
                        start=True,
                        stop=True,
                    )
                last.then_inc(pe_sem)
            # stage 2: transpose m1 128-col blocks into tscr banks
            pe.wait_ge(dve_sem, NK)
            pe.wait_ge(id_sem, 1)
            for t in range(T):
                g_, j = divmod(t, 4)
                if j == 0 and g_ >= 2:
                    pe.wait_ge(act_sem, g_ - 1)
                nblk = 4 if g_ < 4 else 2
                pe.matmul(
                    tscr[g_ % 2][:, j * 128 : (j + 1) * 128],
                    m1[:, t * R : t * R + R],
                    idf32,
                    is_transpose=True,
                    start=(j == 0),
                    stop=(j == nblk - 1),
                ).then_inc(pe_sem)
            # final: partition-sum via ones-column matmul
            pe.wait_ge(act_sem, 6)
            pe.matmul(
                tscr[1][0:1, 0:1], sums, ones, start=True, stop=True
            ).then_inc(pe_sem)

        @block.vector
        def _(v):
            for k in range(NK):
                t, c = divmod(k, NWC)
                v.wait_ge(pe_sem, k + 1)
                v.tensor_reduce(
                    m1[:, t * R + c * 16 : t * R + (c + 1) * 16],
                    pts[k % 2].rearrange("p (g v) -> p g v", v=VR),
                    axis=mybir.AxisListType.X,
                    op=mybir.AluOpType.min,
                ).then_inc(dve_sem)
            # stage 2 tail: v-region min, relu, mask
            v.wait_ge(act_sem, 5)
            v.tensor_reduce(
                md2,
                mts.rearrange("p (r v) -> p r v", v=VR),
                axis=mybir.AxisListType.X,
                op=mybir.AluOpType.min,
            )
            v.drain()
            v.scalar_tensor_tensor(
                md2m,
                md2,
                0.0,
                maskt[:, 0:RH],
                op0=mybir.AluOpType.max,
                op1=mybir.AluOpType.mult,
            ).then_inc(dve_sem)

        @block.scalar
        def _(a):
            # copy transposed blocks PSUM -> SBUF
            for g_ in range(5):
                nblk = 4 if g_ < 4 else 2
                a.wait_ge(pe_sem, NK + g_ * 4 + nblk)
                w = nblk * 128
                a.copy(
                    mts[:, g_ * 512 : g_ * 512 + w], tscr[g_ % 2][:, 0:w]
                ).then_inc(act_sem)
            a.wait_ge(dve_sem, NK + 1)
            a.activation(
                sq48,
                md2m,
                mybir.ActivationFunctionType.Sqrt,
                accum_out=sums,
            ).then_inc(act_sem)
            a.wait_ge(pe_sem, NK + T + 1)
            a.copy(res, tscr[1][0:1, 0:1]).then_inc(act_sem)

    _cache["nc"] = nc
    return nc


def _prep_inputs(v1s, v2s, cmaps, rid_to_vid):
    """Build per-core fp16 fused lhsT|rhs feature matrices + mask."""
    g1 = v1s[:, rid_to_vid, :].astype(np.float16)  # [B, R, VR, 3]
    g2 = v2s[:, rid_to_vid, :].astype(np.float16)
    # squared norms from the same rounded coords (consistent cancellation)
    g1_32 = g1.astype(np.float32)
    g2_32 = g2.astype(np.float32)
    sq1 = (g1_32 * g1_32).sum(-1)  # [B, R, VR]
    sq2 = (g2_32 * g2_32).sum(-1)

    in_maps = []
    for core in range(NCORES):
        b, h = divmod(core, 2)
        rs = slice(RH * h, RH * (h + 1))
        a = np.empty((K, V + W), np.float16)
        a[0:3, 0:V] = -2.0 * g1_32[b, rs].reshape(V, 3).T
        a[3, 0:V] = sq1[b, rs].reshape(V)
        a[4, 0:V] = 1.0
        a[0:3, V:] = g2_32[b].reshape(W, 3).T
        a[3, V:] = 1.0
        a[4, V:] = sq2[b].reshape(W)
        mk = np.empty((R, RH + 1), np.float32)
        # maskt[s, rj] = cmap[b, 24h+rj, s]; col RH = ones
        mk[:, 0:RH] = cmaps[b, rs, :].astype(np.float32).T
        mk[:, RH] = 1.0
        in_maps.append({"ab": a, "maskin": mk})
    return in_maps


def kernel(v1s, v2s, cmaps, rid_to_vid):
    v1s = np.asarray(v1s)
    v2s = np.asarray(v2s)
    cmaps = np.asarray(cmaps)
    rid_to_vid = np.asarray(rid_to_vid)

    nc = _build()
    in_maps = _prep_inputs(v1s, v2s, cmaps, rid_to_vid)
    res = run_bass_kernel_spmd(nc, in_maps, core_ids=list(range(NCORES)))

    counts = cmaps.reshape(B, -1).sum(axis=1).astype(np.float32)
    out = np.empty((B,), np.float32)
    for b in range(B):
        s0 = float(res.results[2 * b]["res_out"][0, 0])
        s1 = float(res.results[2 * b + 1]["res_out"][0, 0])
        out[b] = (s0 + s1) / counts[b]
    return out


# revision 14
# speedup vs baseline: 82.6060x; 82.6060x over previous
"""Trainium2 Bass kernel for nn_ContactMapDistError.

Computes, for each batch element b:
    mean over active contact pairs (r,s) of
      min_{v in region r, w in region s} || g1[b,r,v] - g2[b,s,w] ||

Strategy (v5 — full on-device reduction, scalar output per core,
DVE+Pool split stage-1, DMA-xbar transpose stage-2)
---------------------------------------------------------------
Host (cheap, O(B*R*VR)):
  - gather region vertex subsets g1, g2 via rid_to_vid
  - build fp16 feature matrices so a single K=5 matmul produces the full
    pairwise squared-distance matrix:
        d2(v,w) = [-2x,-2y,-2z,sq1,1]_v . [x',y',z',1,sq2]_w
    (fp16 inputs ~ same precision as the fp32r path: PSUM accumulates f32)
  - mask input [48, 25]: cols 0-23 = cmap slice, col 24 = ones (for the
    final partition-sum matmul)

Device (8 cores, SPMD; core i -> batch i//2, r-half i%2), raw bass:
  - PE: 54 tiles x 3 matmuls [5,128]x[5,512] -> d2 in PSUM [128,1536],
    emitted c-major so compute starts after a partial input DMA
  - stage-1 min over each s-region's 96 w columns -> m1 [128, 1536] fp16
    (column j = 512c + 16t + s_local, the layout the DMA xbar transpose
    wants): 2/3 of tiles on DVE (grouped tensor_reduce from PSUM), 1/3
    copied PSUM->SBUF fp16 by ACT and tree-min'ed on the Pool engine
  - SP: per c-group DMA xbar transpose m1 -> mts[s, 128t+p] [48, 4096],
    overlapped with stage-1
  - DVE: grouped min over each r-region's 96 v columns -> md2 [48, 24];
    relu * mask; ACT: sqrt with free-axis accumulate -> sums [48,1]
  - PE: partition-sum matmul (ones column) -> scalar; DMA out [1,1] f32
"""

import sys

sys.path.insert(0, "/opt/trn_rl_repo")

import numpy as np

import concourse.bass as bass
import concourse.mybir as mybir
from concourse.bass_utils import run_bass_kernel_spmd

F16 = mybir.dt.float16
F32 = mybir.dt.float32

B, N, R, VR = 4, 10475, 48, 96
NCORES = 8
RH = R // 2            # r-regions handled per core
V = RH * VR            # packed v columns per core = 2304
T = V // 128           # v-chunks of 128 partitions = 18
TS = 32                # t-slots per s in the m1 layout (14 padding)
W = R * VR             # full w width = 4608
WC = 1536              # psum w-chunk (3 banks, 16 s-regions)
NWC = W // WC          # = 3
K = 5                  # contraction dim
NK = T * NWC           # stage-1 tile count = 54
POOL_EVERY = 10**9     # Pool TensorTensor is illegal on TRN2 walrus; DVE-only
NSTAGE = 3             # SBUF staging depth for the Pool path

_cache = {}


def _pool_tile(k):
    """Stage-1 tiles routed to the ACT+Pool path (rest go to DVE)."""
    return k % POOL_EVERY == POOL_EVERY - 1


def _build():
    if "nc" in _cache:
        return _cache["nc"]
    nc = bass.Bass()
    ab = nc.declare_dram_parameter("ab", [K, V + W], F16, isOutput=False)
    maskin = nc.declare_dram_parameter("maskin", [2 * R, RH + 1], F32, isOutput=False)
    res_out = nc.declare_dram_parameter("res_out", [1, 1], F32, isOutput=True)

    abt = nc.alloc_sbuf_tensor("abt", [K, V + W], F16).ap()
    maskt = nc.alloc_sbuf_tensor("maskt", [2 * R, RH + 1], F32).ap()
    m1 = nc.alloc_sbuf_tensor("m1", [128, NWC * T * 128], F16).ap()
    mtsb = [
        nc.alloc_sbuf_tensor(f"mts{i}", [128, T * 128], F16).ap()
        for i in range(NWC)
    ]
    md2 = nc.alloc_sbuf_tensor("md2", [2 * R, RH], F32).ap()
    md2m = nc.alloc_sbuf_tensor("md2m", [2 * R, RH], F32).ap()
    sq48 = nc.alloc_sbuf_tensor("sq48", [2 * R, RH], F32).ap()
    sums = nc.alloc_sbuf_tensor("sums", [2 * R, 1], F32).ap()
    res = nc.alloc_sbuf_tensor("res", [1, 1], F32).ap()
    pts = [nc.alloc_psum_tensor(f"pt{i}", [128, WC], F32).ap() for i in range(2)]
    fres = nc.alloc_psum_tensor("fres", [1, 1], F32).ap()

    lt = abt[:, 0:V]          # v-side features (stationary)
    rt = abt[:, V : V + W]    # w-side features (moving)
    ones = maskt[:, RH : RH + 1]

    # tile k = c * T + t  (c-major emission)
    def tile_tc(k):
        c, t = divmod(k, T)
        return t, c

    # m1 output slice for tile k: cols [2304c + 128t, +16)
    def m1_cols(k):
        t, c = tile_tc(k)
        off = T * 128 * c + 128 * t
        return m1[:, off : off + 16]

    # per-tile consumer bookkeeping for psum double-buffer reuse
    ndve = [0] * (NK + 1)  # ndve[k] = #dve-routed tiles among 0..k-1
    npool = [0] * (NK + 1)
    for k in range(NK):
        ndve[k + 1] = ndve[k] + (0 if _pool_tile(k) else 1)
        npool[k + 1] = npool[k] + (1 if _pool_tile(k) else 0)
    NDVE, NPOOL = ndve[NK], npool[NK]

    with (
        nc.Block() as block,
        nc.semaphore("dma_sem") as dma_sem,
        nc.semaphore("dmb_sem") as dmb_sem,
        nc.semaphore("tdma_sem") as tdma_sem,
        nc.semaphore("pe_sem") as pe_sem,
        nc.semaphore("dve_c") as dve_c,
        nc.semaphore("acop_c") as acop_c,
        nc.semaphore("pool_c") as pool_c,
        nc.semaphore("fin_sem") as fin_sem,
        nc.semaphore("act_sem") as act_sem,
    ):

        @block.sync
        def _(sp):
            sp.dma_start(maskt, maskin[:]).then_inc(dma_sem, 16)
            # A: v-side features + first w-chunk -> compute starts early
            sp.dma_start(
                abt[:, 0 : V + WC], ab[:, 0 : V + WC]
            ).then_inc(dma_sem, 16)
            sp.dma_start(
                abt[:, V + WC : V + W], ab[:, V + WC : V + W]
            ).then_inc(dmb_sem, 16)
            # stage 2: xbar transpose per completed c-group
            for c in range(NWC):
                sp.wait_ge(dve_c, ndve[(c + 1) * T])
                sp.dma_start_transpose(
                    mtsb[c].rearrange("r (q p) -> r q p", p=128),
                    m1[:, T * 128 * c : T * 128 * (c + 1)],
                ).then_inc(tdma_sem, 16)
            sp.wait_ge(act_sem, 2)
            sp.dma_start(res_out[:], res).then_inc(dma_sem, 16)
            sp.wait_ge(dma_sem, 48)
            sp.wait_ge(dmb_sem, 16)

        @block.tensor
        def _(pe):
            pe.wait_ge(dma_sem, 32)
            pe.wait_ge(pool_c, 1)
            # stage 1: d2 tiles (c-major)
            for k in range(NK):
                t, c = tile_tc(k)
                if k == T:
                    pe.wait_ge(dmb_sem, 16)
                prev = k - 2
                if prev >= 0:
                    # previous occupant of pts[k%2] must be consumed
                    if _pool_tile(prev):
                        pe.wait_ge(acop_c, npool[prev + 1])
                    else:
                        pe.wait_ge(dve_c, ndve[prev + 1])
                pt = pts[k % 2]
                last = None
                for m in range(WC // 512):
                    last = pe.matmul(
                        pt[:, m * 512 : (m + 1) * 512],
                        lt[:, t * 128 : (t + 1) * 128],
                        rt[:, c * WC + m * 512 : c * WC + (m + 1) * 512],
                        start=True,
                        stop=True,
                    )
                last.then_inc(pe_sem)
            # final: partition-sum via ones-column matmul
            pe.wait_ge(act_sem, 1)
            pe.matmul(fres, sums, ones, start=True, stop=True).then_inc(pe_sem)

        @block.vector
        def _(v):
            v.memset(md2, 0)
            for k in range(NK):
                if _pool_tile(k):
                    continue
                v.wait_ge(pe_sem, k + 1)
                v.tensor_reduce(
                    m1_cols(k),
                    pts[k % 2].rearrange("p (g v) -> p g v", v=VR),
                    axis=mybir.AxisListType.X,
                    op=mybir.AluOpType.min,
                ).then_inc(dve_c)
            # stage 2 tail: v-region min, relu, mask
            v.wait_ge(tdma_sem, NWC * 16)
            for c in range(NWC):
                v.tensor_reduce(
                    md2[32 * c : 32 * c + 16, :],
                    mtsb[c][0:16, 0:V].rearrange("p (r v) -> p r v", v=VR),
                    axis=mybir.AxisListType.X,
                    op=mybir.AluOpType.min,
                )
            v.drain()
            v.scalar_tensor_tensor(
                md2m,
                md2,
                0.0,
                maskt[:, 0:RH],
                op0=mybir.AluOpType.max,
                op1=mybir.AluOpType.mult,
            ).then_inc(fin_sem)

        @block.gpsimd
        def _(g):
            # zero m1 (the xbar transpose reads the 112 pad cols per block);
            # uint32 view halves the element count
            g.memset(m1.bitcast(mybir.dt.uint32), 0)
            g.drain().then_inc(pool_c)

        @block.scalar
        def _(a):
            # stage-1 assist: copy pool-routed tiles PSUM -> SBUF fp16
            i = 0
            for k in range(NK):
                if not _pool_tile(k):
                    continue
                a.wait_ge(pe_sem, k + 1)
                if i >= NSTAGE:
                    a.wait_ge(pool_c, i - NSTAGE + 1)
                a.copy(psta[i % NSTAGE], pts[k % 2]).then_inc(acop_c)
                i += 1
            a.wait_ge(fin_sem, 1)
            a.activation(
                sq48,
                md2m,
                mybir.ActivationFunctionType.Sqrt,
                accum_out=sums,
            ).then_inc(act_sem)
            a.wait_ge(pe_sem, NK + 1)
            a.copy(res, fres).then_inc(act_sem)

    _cache["nc"] = nc
    return nc


def _prep_inputs(v1s, v2s, cmaps, rid_to_vid):
    """Build per-core fp16 fused lhsT|rhs feature matrices + mask."""
    g1 = v1s[:, rid_to_vid, :].astype(np.float16)  # [B, R, VR, 3]
    g2 = v2s[:, rid_to_vid, :].astype(np.float16)
    # squared norms from the same rounded coords (consistent cancellation)
    g1_32 = g1.astype(np.float32)
    g2_32 = g2.astype(np.float32)
    sq1 = (g1_32 * g1_32).sum(-1)  # [B, R, VR]
    sq2 = (g2_32 * g2_32).sum(-1)

    in_maps = []
    for core in range(NCORES):
        b, h = divmod(core, 2)
        rs = slice(RH * h, RH * (h + 1))
        a = np.empty((K, V + W), np.float16)
        a[0:3, 0:V] = -2.0 * g1_32[b, rs].reshape(V, 3).T
        a[3, 0:V] = sq1[b, rs].reshape(V)
        a[4, 0:V] = 1.0
        a[0:3, V:] = g2_32[b].reshape(W, 3).T
        a[3, V:] = 1.0
        a[4, V:] = sq2[b].reshape(W)
        mk = np.zeros((2 * R, RH + 1), np.float32)
        # row 32c+j holds s = 16c+j: maskt[row, rj] = cmap[b, 24h+rj, s]
        cm = cmaps[b, rs, :].astype(np.float32).T  # [s, rj]
        for c3 in range(3):
            mk[32 * c3 : 32 * c3 + 16, 0:RH] = cm[16 * c3 : 16 * (c3 + 1)]
        mk[:, RH] = 1.0
        in_maps.append({"ab": a, "maskin": mk})
    return in_maps


def kernel(v1s, v2s, cmaps, rid_to_vid):
    v1s = np.asarray(v1s)
    v2s = np.asarray(v2s)
    cmaps = np.asarray(cmaps)
    rid_to_vid = np.asarray(rid_to_vid)

    nc = _build()
    in_maps = _prep_inputs(v1s, v2s, cmaps, rid_to_vid)
    res = run_bass_kernel_spmd(nc, in_maps, core_ids=list(range(NCORES)))

    counts = cmaps.reshape(B, -1).sum(axis=1).astype(np.float32)
    out = np.empty((B,), np.float32)
    for b in range(B):
        s0 = float(res.results[2 * b]["res_out"][0, 0])
        s1 = float(res.results[2 * b + 1]["res_out"][0, 0])
        out[b] = (s0 + s1) / counts[b]
    return out


# revision 15
# speedup vs baseline: 97.8522x; 1.1846x over previous
"""Trainium2 Bass kernel for nn_ContactMapDistError.

Computes, for each batch element b:
    mean over active contact pairs (r,s) of
      min_{v in region r, w in region s} || g1[b,r,v] - g2[b,s,w] ||

Strategy (full on-device reduction, scalar output per core)
-----------------------------------------------------------
Host (cheap, O(B*R*VR)):
  - gather region vertex subsets g1, g2 via rid_to_vid
  - build fp16 feature matrices so a single K=5 matmul produces the full
    pairwise squared-distance matrix:
        d2(v,w) = [-2x,-2y,-2z,sq1,1]_v . [x',y',z',1,sq2]_w
    (fp16 inputs ~ same precision as the fp32r path: PSUM accumulates f32)
  - mask input [48, 25]: cols 0-23 = cmap slice, col 24 = ones (for the
    final partition-sum matmul)

Device (8 cores, SPMD; core i -> batch i//2, r-half i%2), raw bass:
  - PE: 54 tiles x 3 matmuls [5,128]x[5,512] -> d2 in PSUM [128,1536]
  - DVE: grouped min over each s-region's 96 w columns -> m1 [128, 18*48]
  - PE: 18 transposes (identity built on Pool via iota+is_equal) to flip
    the packed v axis into the free dimension; ACT copies to mts [48,2304]
  - DVE: grouped min over each r-region's 96 v columns -> md2 [48, 24]
  - DVE: relu * mask; ACT: sqrt with free-axis accumulate -> sums [48,1]
  - PE: partition-sum matmul (ones column) -> scalar; DMA out [1,1] f32

The output per core is a single f32 (the masked sum of min distances for
this core's (r-half, all s) block); the host divides by the contact count.
This keeps per-invocation I/O tiny: ~74 KB in, 4 B out per core.
"""

import sys

sys.path.insert(0, "/opt/trn_rl_repo")

import numpy as np

import concourse.bass as bass
import concourse.mybir as mybir
from concourse.bass_utils import run_bass_kernel_spmd

F16 = mybir.dt.float16
F32 = mybir.dt.float32
I32 = mybir.dt.int32

B, N, R, VR = 4, 10475, 48, 96
NCORES = 8
RH = R // 2            # r-regions handled per core
V = RH * VR            # packed v columns per core = 2304
T = V // 128           # v-chunks of 128 partitions = 18
W = R * VR             # full w width = 4608
WC = 1536              # psum w-chunk (3 banks, 16 s-regions)
NWC = W // WC          # = 3
K = 5                  # contraction dim
NK = T * NWC           # stage-1 tile count = 54

_cache = {}


def _build():
    if "nc" in _cache:
        return _cache["nc"]
    nc = bass.Bass()
    ab = nc.declare_dram_parameter("ab", [K, V + W], F16, isOutput=False)
    maskin = nc.declare_dram_parameter("maskin", [R, RH + 1], F32, isOutput=False)
    res_out = nc.declare_dram_parameter("res_out", [1, 1], F32, isOutput=True)

    abt = nc.alloc_sbuf_tensor("abt", [K, V + W], F16).ap()
    maskt = nc.alloc_sbuf_tensor("maskt", [R, RH + 1], F32).ap()
    iot = nc.alloc_sbuf_tensor("iot", [128, 128], I32).ap()
    idf32 = nc.alloc_sbuf_tensor("idf32", [128, 128], F32).ap()
    m1 = nc.alloc_sbuf_tensor("m1", [128, T * R], F32).ap()
    mts = nc.alloc_sbuf_tensor("mts", [R, V], F32).ap()
    md2 = nc.alloc_sbuf_tensor("md2", [R, RH], F32).ap()
    md2m = nc.alloc_sbuf_tensor("md2m", [R, RH], F32).ap()
    sq48 = nc.alloc_sbuf_tensor("sq48", [R, RH], F32).ap()
    sums = nc.alloc_sbuf_tensor("sums", [R, 1], F32).ap()
    res = nc.alloc_sbuf_tensor("res", [1, 1], F32).ap()

    pts = [nc.alloc_psum_tensor(f"pt{i}", [128, WC], F32).ap() for i in range(2)]
    tscr = [nc.alloc_psum_tensor(f"ts{i}", [R, 512], F32).ap() for i in range(2)]

    lt = abt[:, 0:V]          # v-side features (stationary)
    rt = abt[:, V : V + W]    # w-side features (moving)
    ones = maskt[:, RH : RH + 1]

    with (
        nc.Block() as block,
        nc.semaphore("dma_sem") as dma_sem,
        nc.semaphore("pe_sem") as pe_sem,
        nc.semaphore("dve_sem") as dve_sem,
        nc.semaphore("act_sem") as act_sem,
        nc.semaphore("id_sem") as id_sem,
    ):

        @block.sync
        def _(sp):
            sp.dma_start(abt, ab[:]).then_inc(dma_sem, 16)
            sp.dma_start(maskt, maskin[:]).then_inc(dma_sem, 16)
            sp.wait_ge(act_sem, 7)
            sp.dma_start(res_out[:], res).then_inc(dma_sem, 16)
            sp.wait_ge(dma_sem, 48)

        @block.gpsimd
        def _(g):
            # identity matrix for PE transposes: (j - p == 0)
            g.iota(
                iot,
                [[1, 128]],
                base=0,
                channel_multiplier=-1,
                allow_small_or_imprecise_dtypes=True,
            )
            g.drain()
            g.tensor_scalar(
                idf32, iot, 0, None, op0=mybir.AluOpType.is_equal
            ).then_inc(id_sem)

        @block.tensor
        def _(pe):
            pe.wait_ge(dma_sem, 32)
            # stage 1: d2 tiles
            for k in range(NK):
                t, c = divmod(k, NWC)
                if k >= 2:
                    pe.wait_ge(dve_sem, k - 1)
                pt = pts[k % 2]
                last = None
                for m in range(WC // 512):
                    last = pe.matmul(
                        pt[:, m * 512 : (m + 1) * 512],
                        lt[:, t * 128 : (t + 1) * 128],
                        rt[:, c * WC + m * 512 : c * WC + (m + 1) * 512],
                        start=True,
                        stop=True,
                    )
                last.then_inc(pe_sem)
            # stage 2: transpose m1 128-col blocks into tscr banks
            pe.wait_ge(dve_sem, NK)
            pe.wait_ge(id_sem, 1)
            for t in range(T):
                g_, j = divmod(t, 4)
                if j == 0 and g_ >= 2:
                    pe.wait_ge(act_sem, g_ - 1)
                nblk = 4 if g_ < 4 else 2
                pe.matmul(
                    tscr[g_ % 2][:, j * 128 : (j + 1) * 128],
                    m1[:, t * R : t * R + R],
                    idf32,
                    is_transpose=True,
                    start=(j == 0),
                    stop=(j == nblk - 1),
                ).then_inc(pe_sem)
            # final: partition-sum via ones-column matmul
            pe.wait_ge(act_sem, 6)
            pe.matmul(
                tscr[1][0:1, 0:1], sums, ones, start=True, stop=True
            ).then_inc(pe_sem)

        @block.vector
        def _(v):
            for k in range(NK):
                t, c = divmod(k, NWC)
                v.wait_ge(pe_sem, k + 1)
                v.tensor_reduce(
                    m1[:, t * R + c * 16 : t * R + (c + 1) * 16],
                    pts[k % 2].rearrange("p (g v) -> p g v", v=VR),
                    axis=mybir.AxisListType.X,
                    op=mybir.AluOpType.min,
                ).then_inc(dve_sem)
            # stage 2 tail: v-region min, relu, mask
            v.wait_ge(act_sem, 5)
            v.tensor_reduce(
                md2,
                mts.rearrange("p (r v) -> p r v", v=VR),
                axis=mybir.AxisListType.X,
                op=mybir.AluOpType.min,
            )
            v.drain()
            v.scalar_tensor_tensor(
                md2m,
                md2,
                0.0,
                maskt[:, 0:RH],
                op0=mybir.AluOpType.max,
                op1=mybir.AluOpType.mult,
            ).then_inc(dve_sem)

        @block.scalar
        def _(a):
            # copy transposed blocks PSUM -> SBUF
            for g_ in range(5):
                nblk = 4 if g_ < 4 else 2
                a.wait_ge(pe_sem, NK + g_ * 4 + nblk)
                w = nblk * 128
                a.copy(
                    mts[:, g_ * 512 : g_ * 512 + w], tscr[g_ % 2][:, 0:w]
                ).then_inc(act_sem)
            a.wait_ge(dve_sem, NK + 1)
            a.activation(
                sq48,
                md2m,
                mybir.ActivationFunctionType.Sqrt,
                accum_out=sums,
            ).then_inc(act_sem)
            a.wait_ge(pe_sem, NK + T + 1)
            a.copy(res, tscr[1][0:1, 0:1]).then_inc(act_sem)

    _cache["nc"] = nc
    return nc


def _prep_inputs(v1s, v2s, cmaps, rid_to_vid):
    """Build per-core fp16 fused lhsT|rhs feature matrices + mask."""
    g1 = v1s[:, rid_to_vid, :].astype(np.float16)  # [B, R, VR, 3]
    g2 = v2s[:, rid_to_vid, :].astype(np.float16)
    # squared norms from the same rounded coords (consistent cancellation)
    g1_32 = g1.astype(np.float32)
    g2_32 = g2.astype(np.float32)
    sq1 = (g1_32 * g1_32).sum(-1)  # [B, R, VR]
    sq2 = (g2_32 * g2_32).sum(-1)

    in_maps = []
    for core in range(NCORES):
        b, h = divmod(core, 2)
        rs = slice(RH * h, RH * (h + 1))
        a = np.empty((K, V + W), np.float16)
        a[0:3, 0:V] = -2.0 * g1_32[b, rs].reshape(V, 3).T
        a[3, 0:V] = sq1[b, rs].reshape(V)
        a[4, 0:V] = 1.0
        a[0:3, V:] = g2_32[b].reshape(W, 3).T
        a[3, V:] = 1.0
        a[4, V:] = sq2[b].reshape(W)
        mk = np.empty((R, RH + 1), np.float32)
        # maskt[s, rj] = cmap[b, 24h+rj, s]; col RH = ones
        mk[:, 0:RH] = cmaps[b, rs, :].astype(np.float32).T
        mk[:, RH] = 1.0
        in_maps.append({"ab": a, "maskin": mk})
    return in_maps


def kernel(v1s, v2s, cmaps, rid_to_vid):
    v1s = np.asarray(v1s)
    v2s = np.asarray(v2s)
    cmaps = np.asarray(cmaps)
    rid_to_vid = np.asarray(rid_to_vid)

    nc = _build()
    in_maps = _prep_inputs(v1s, v2s, cmaps, rid_to_vid)
    res = run_bass_kernel_spmd(nc, in_maps, core_ids=list(range(NCORES)))

    counts = cmaps.reshape(B, -1).sum(axis=1).astype(np.float32)
    out = np.empty((B,), np.float32)
    for b in range(B):
        s0 = float(res.results[2 * b]["res_out"][0, 0])
        s1 = float(res.results[2 * b + 1]["res_out"][0, 0])
        out[b] = (s0 + s1) / counts[b]
    return out
